# revision 2
# baseline (speedup 1.0000x reference)
import numpy as np
import ml_dtypes
from contextlib import ExitStack

import concourse.bacc as bacc
import concourse.bass as bass
import concourse.tile as tile
import concourse.mybir as mybir
from concourse import bass_utils

dt = mybir.dt
AF = mybir.ActivationFunctionType
F32 = dt.float32
BF16 = dt.bfloat16

B, H, NH, NKV, HD, MAXLEN = 64, 2048, 16, 2, 128, 8192
NCORES = 8
BL = 16
NHL = 8
NT = MAXLEN // 128
CH = NT // 4
KT = H // 128
SCALE = 1.0 / np.sqrt(HD)

_CACHE = {}


def _build():
    nc = bacc.Bacc("TRN2", target_bir_lowering=False, debug=False)

    def din(name, shape):
        return nc.dram_tensor(name, shape, F32, kind="ExternalInput")

    x_til = din("x_til", (128, KT, BL))
    wq_til = din("wq_til", (KT * 128, NHL * 128))
    bq4 = din("bq4", (BL, NHL, 2, 64))
    wk_til = din("wk_til", (KT * 128, 128))
    bk4 = din("bk4", (BL, 2, 64))
    wv_til = din("wv_til", (KT * 128, 128))
    bv4 = din("bv4", (BL, 2, 64))
    wo_til = din("wo_til", (NHL * 128, H))
    cosq = din("cosq", (BL, NHL, 2, 64))
    sinq = din("sinq", (BL, NHL, 2, 64))
    cosk = din("cosk", (BL, 2, 64))
    sink = din("sink", (BL, 2, 64))
    ck = din("ck", (BL * 128, NT, 128))
    cv = din("cv", (BL * 128, NT, 128))
    idf = din("idf", (128, 128))
    idb = nc.dram_tensor("idb", (128, 128), BF16, kind="ExternalInput")
    ones1 = din("ones1", (1, 128))
    out_d = nc.dram_tensor("out", (BL, H), F32, kind="ExternalOutput")

    with tile.TileContext(nc) as tc, ExitStack() as ctx:
        cst = ctx.enter_context(tc.tile_pool(name="cst", bufs=1))
        idf_sb = cst.tile([128, 128], F32, tag="idf")
        nc.sync.dma_start(idf_sb[:], idf.ap())
        idb_sb = cst.tile([128, 128], BF16, tag="idb")
        nc.sync.dma_start(idb_sb[:], idb.ap())
        ones_sb = cst.tile([1, 128], F32, tag="ones")
        nc.sync.dma_start(ones_sb[:], ones1.ap())
        x_sb = cst.tile([128, KT, BL], F32, tag="x")
        nc.sync.dma_start(x_sb[:], x_til.ap())
        cosq_sb = cst.tile([BL, NHL, 2, 64], F32, tag="cq")
        nc.sync.dma_start(cosq_sb[:], cosq.ap())
        sinq_sb = cst.tile([BL, NHL, 2, 64], F32, tag="sq")
        nc.sync.dma_start(sinq_sb[:], sinq.ap())
        cosk_sb = cst.tile([BL, 2, 64], F32, tag="ck_")
        nc.sync.dma_start(cosk_sb[:], cosk.ap())
        sink_sb = cst.tile([BL, 2, 64], F32, tag="sk_")
        nc.sync.dma_start(sink_sb[:], sink.ap())
        bq_sb = cst.tile([BL, NHL, 2, 64], F32, tag="bq")
        nc.sync.dma_start(bq_sb[:], bq4.ap())
        bk_sb = cst.tile([BL, 2, 64], F32, tag="bk")
        nc.sync.dma_start(bk_sb[:], bk4.ap())
        bv_sb = cst.tile([BL, 2, 64], F32, tag="bv")
        nc.sync.dma_start(bv_sb[:], bv4.ap())

        per = ctx.enter_context(tc.tile_pool(name="per", bufs=1))
        qT_sb = per.tile([128, NHL, BL], BF16, tag="qT")
        kr_sb = per.tile([BL, 2, 64], F32, tag="kr")
        v_sb = per.tile([BL, 2, 64], F32, tag="vn")
        attnT_sb = per.tile([128, NHL, BL], F32, tag="attnT")

        with tc.tile_pool(name="wp", bufs=3) as wp, \
             tc.tile_pool(name="pps", bufs=1, space="PSUM") as pps, \
             tc.tile_pool(name="prj", bufs=1) as prj:
            qp0 = pps.tile([BL, 4, 2, 64], F32, tag="qp0")
            qp1 = pps.tile([BL, 4, 2, 64], F32, tag="qp1")
            kp = pps.tile([BL, 2, 64], F32, tag="kp")
            vp = pps.tile([BL, 2, 64], F32, tag="vp")
            for kt in range(KT):
                wq_sb = wp.tile([128, NHL * 128], F32, tag="wq")
                nc.sync.dma_start(wq_sb[:], wq_til.ap()[kt * 128:(kt + 1) * 128])
                wk_sb = wp.tile([128, 128], F32, tag="wk")
                nc.sync.dma_start(wk_sb[:], wk_til.ap()[kt * 128:(kt + 1) * 128])
                wv_sb = wp.tile([128, 128], F32, tag="wv")
                nc.sync.dma_start(wv_sb[:], wv_til.ap()[kt * 128:(kt + 1) * 128])
                lhs = x_sb[:, kt, :]
                st, sp_ = kt == 0, kt == KT - 1
                nc.tensor.matmul(qp0[:], lhs, wq_sb[:, 0:512], start=st, stop=sp_)
                nc.tensor.matmul(qp1[:], lhs, wq_sb[:, 512:1024], start=st, stop=sp_)
                nc.tensor.matmul(kp[:], lhs, wk_sb[:], start=st, stop=sp_)
                nc.tensor.matmul(vp[:], lhs, wv_sb[:], start=st, stop=sp_)

            q_sb = prj.tile([BL, NHL, 2, 64], F32, tag="q")
            nc.vector.tensor_add(q_sb[:, 0:4, :, :], qp0[:], bq_sb[:, 0:4, :, :])
            nc.vector.tensor_add(q_sb[:, 4:8, :, :], qp1[:], bq_sb[:, 4:8, :, :])
            k_sb = prj.tile([BL, 2, 64], F32, tag="k")
            nc.vector.tensor_add(k_sb[:], kp[:], bk_sb[:])
            nc.vector.tensor_add(v_sb[:], vp[:], bv_sb[:])

            tmp = prj.tile([BL, NHL, 2, 64], F32, tag="tq")
            nc.vector.tensor_mul(tmp[:, :, 0, :], q_sb[:, :, 1, :], sinq_sb[:, :, 0, :])
            nc.vector.tensor_mul(tmp[:, :, 1, :], q_sb[:, :, 0, :], sinq_sb[:, :, 1, :])
            qr = prj.tile([BL, NHL, 2, 64], F32, tag="qr")
            nc.vector.tensor_mul(qr[:], q_sb[:], cosq_sb[:])
            nc.vector.tensor_add(qr[:], qr[:], tmp[:])
            tk = prj.tile([BL, 2, 64], F32, tag="tk")
            nc.vector.tensor_mul(tk[:, 0, :], k_sb[:, 1, :], sink_sb[:, 0, :])
            nc.vector.tensor_mul(tk[:, 1, :], k_sb[:, 0, :], sink_sb[:, 1, :])
            nc.vector.tensor_mul(kr_sb[:], k_sb[:], cosk_sb[:])
            nc.vector.tensor_add(kr_sb[:], kr_sb[:], tk[:])

            for h in range(NHL):
                qtp = pps.tile([128, BL], F32, tag="qtp")
                nc.tensor.matmul(qtp[:], qr[:, h, :, :], idf_sb[0:BL, 0:BL],
                                 start=True, stop=True, is_transpose=True)
                nc.vector.tensor_copy(qT_sb[:, h, :], qtp[:])

        with tc.tile_pool(name="kv", bufs=2) as kvp, \
             tc.tile_pool(name="kvb", bufs=2) as kvbp, \
             tc.tile_pool(name="att", bufs=3) as attp, \
             tc.tile_pool(name="kt_ps", bufs=2, space="PSUM") as kt_ps, \
             tc.tile_pool(name="s_ps", bufs=1, space="PSUM") as s_ps, \
             tc.tile_pool(name="pt_ps", bufs=2, space="PSUM") as pt_ps, \
             tc.tile_pool(name="ot_ps", bufs=2, space="PSUM") as ot_ps, \
             tc.tile_pool(name="m_ps", bufs=1, space="PSUM") as m_ps:
            for b in range(BL):
                kk = kvp.tile([128, NT, 128], F32, tag="kk")
                nc.sync.dma_start(kk[:], ck.ap()[b * 128:(b + 1) * 128])
                vv = kvp.tile([128, NT, 128], F32, tag="vv")
                nc.sync.dma_start(vv[:], cv.ap()[b * 128:(b + 1) * 128])
                nc.sync.dma_start(kk[127:128, NT - 1, :], kr_sb[b:b + 1, :, :])
                nc.sync.dma_start(vv[127:128, NT - 1, :], v_sb[b:b + 1, :, :])
                vb = kvbp.tile([128, NT, 128], BF16, tag="vb")
                nc.scalar.activation(vb[:, 0:NT // 2, :], vv[:, 0:NT // 2, :], AF.Copy)
                nc.scalar.activation(vb[:, NT // 2:NT, :], vv[:, NT // 2:NT, :], AF.Copy)

                sums = attp.tile([NHL, CH], F32, tag="sums")
                ot = ot_ps.tile([128, NHL], F32, tag="ot")
                qtb = qT_sb[:, :, b]
                for c in range(CH):
                    ktp = kt_ps.tile([128, 512], F32, tag="ktp")
                    for j in range(4):
                        nc.tensor.matmul(ktp[:, j * 128:(j + 1) * 128],
                                         kk[:, c * 4 + j, :], idf_sb[:],
                                         start=(j == 0), stop=(j == 3),
                                         is_transpose=True)
                    kt_sb = attp.tile([128, 512], BF16, tag="ktsb")
                    nc.vector.tensor_copy(kt_sb[:], ktp[:])
                    sp = s_ps.tile([NHL, 512], F32, tag="sp")
                    nc.tensor.matmul(sp[:], qtb, kt_sb[:], start=True, stop=True)
                    pb = attp.tile([NHL, 512], BF16, tag="pb")
                    nc.scalar.activation(pb[:], sp[:], AF.Exp,
                                         accum_out=sums[:, c:c + 1])
                    ptp = pt_ps.tile([128, 4 * NHL], BF16, tag="ptp")
                    for j in range(4):
                        nc.tensor.matmul(ptp[:, j * NHL:(j + 1) * NHL],
                                         pb[:, j * 128:(j + 1) * 128],
                                         idb_sb[0:NHL, 0:NHL],
                                         start=(j == 0), stop=(j == 3),
                                         is_transpose=True)
                    pt_sb = attp.tile([128, 4 * NHL], BF16, tag="ptsb")
                    nc.vector.tensor_copy(pt_sb[:], ptp[:])
                    for j in range(4):
                        i = c * 4 + j
                        nc.tensor.matmul(ot[:], vb[:, i, :],
                                         pt_sb[:, j * NHL:(j + 1) * NHL],
                                         start=(i == 0), stop=(i == NT - 1))

                s8 = attp.tile([NHL, 1], F32, tag="s8")
                nc.vector.tensor_reduce(s8[:], sums[:], mybir.AxisListType.X,
                                        mybir.AluOpType.add)
                r8 = attp.tile([NHL, 1], F32, tag="r8")
                nc.vector.reciprocal(r8[:], s8[:])
                rtp = m_ps.tile([1, NHL], F32, tag="mps")
                nc.tensor.matmul(rtp[:], r8[:], idf_sb[0:NHL, 0:NHL],
                                 start=True, stop=True, is_transpose=True)
                rt_sb = attp.tile([1, NHL], F32, tag="rtsb")
                nc.vector.tensor_copy(rt_sb[:], rtp[:])
                rbp = m_ps.tile([128, NHL], F32, tag="mps")
                nc.tensor.matmul(rbp[:], ones_sb[:], rt_sb[:], start=True, stop=True)
                rb_sb = attp.tile([128, NHL], F32, tag="rbsb")
                nc.vector.tensor_copy(rb_sb[:], rbp[:])
                nc.vector.tensor_mul(attnT_sb[:, :, b], ot[:], rb_sb[:])

        with tc.tile_pool(name="wo", bufs=2) as wop, \
             tc.tile_pool(name="ops", bufs=1, space="PSUM") as ops, \
             tc.tile_pool(name="osb", bufs=1) as osb:
            opt = []
            for n in range(4):
                o_t = ops.tile([BL, 512], F32, tag=f"op{n}", name=f"op{n}")
                opt.append(o_t)
            for h in range(NHL):
                wo_sb = wop.tile([128, H], F32, tag="wo")
                nc.sync.dma_start(wo_sb[:], wo_til.ap()[h * 128:(h + 1) * 128])
                for n in range(4):
                    nc.tensor.matmul(opt[n][:], attnT_sb[:, h, :],
                                     wo_sb[:, n * 512:(n + 1) * 512],
                                     start=(h == 0), stop=(h == NHL - 1))
            out_sb = osb.tile([BL, H], F32, tag="outsb")
            for n in range(4):
                nc.vector.tensor_copy(out_sb[:, n * 512:(n + 1) * 512], opt[n][:])
            nc.sync.dma_start(out_d.ap(), out_sb[:])

    nc.compile()
    return nc


def _prep_core(g, q, x, cos, sin, wq, bq, wk, bk, wv, bv, wo, cache_k, cache_v):
    bs = slice(q * BL, (q + 1) * BL)
    hs = slice(g * NHL * HD, (g + 1) * NHL * HD)
    ks = slice(g * HD, (g + 1) * HD)
    f = np.float32

    xT = np.ascontiguousarray(x[bs, 0, :].T, dtype=f)
    x_til = np.ascontiguousarray(xT.reshape(KT, 128, BL).transpose(1, 0, 2))
    wq_til = np.ascontiguousarray(wq[hs, :].T, dtype=f)
    wk_til = np.ascontiguousarray(wk[ks, :].T, dtype=f)
    wv_til = np.ascontiguousarray(wv[ks, :].T, dtype=f)
    wo_til = np.ascontiguousarray(wo[:, hs].T, dtype=f)

    bq4 = np.broadcast_to(bq[hs], (BL, NHL * HD)).reshape(BL, NHL, 2, 64)
    bk4 = np.broadcast_to(bk[ks], (BL, HD)).reshape(BL, 2, 64)
    bv4 = np.broadcast_to(bv[ks], (BL, HD)).reshape(BL, 2, 64)

    c = cos[0].astype(f)
    s = sin[0].astype(f)
    ssg = np.concatenate([-s[64:], s[:64]])
    sin_signed = np.concatenate([-s[:64], s[64:]])
    cosq = np.broadcast_to(np.tile(c * SCALE, NHL), (BL, NHL * HD)).reshape(BL, NHL, 2, 64)
    sinq = np.broadcast_to(np.tile(sin_signed * SCALE, NHL), (BL, NHL * HD)).reshape(BL, NHL, 2, 64)
    cosk = np.broadcast_to(c, (BL, HD)).reshape(BL, 2, 64)
    sink = np.broadcast_to(sin_signed, (BL, HD)).reshape(BL, 2, 64)
    del ssg

    ckl = cache_k[bs, :, g, :]
    ck = np.ascontiguousarray(
        ckl.reshape(BL, NT, 128, 128).transpose(0, 2, 1, 3), dtype=f
    ).reshape(BL * 128, NT, 128)
    cvl = cache_v[bs, :, g, :]
    cv = np.ascontiguousarray(
        cvl.reshape(BL, NT, 128, 128).transpose(0, 2, 1, 3), dtype=f
    ).reshape(BL * 128, NT, 128)

    return {
        "x_til": x_til, "wq_til": wq_til, "bq4": np.ascontiguousarray(bq4, dtype=f),
        "wk_til": wk_til, "bk4": np.ascontiguousarray(bk4, dtype=f),
        "wv_til": wv_til, "bv4": np.ascontiguousarray(bv4, dtype=f),
        "wo_til": wo_til,
        "cosq": np.ascontiguousarray(cosq, dtype=f),
        "sinq": np.ascontiguousarray(sinq, dtype=f),
        "cosk": np.ascontiguousarray(cosk, dtype=f),
        "sink": np.ascontiguousarray(sink, dtype=f),
        "ck": ck, "cv": cv,
        "idf": np.eye(128, dtype=f),
        "idb": np.eye(128, dtype=ml_dtypes.bfloat16),
        "ones1": np.ones((1, 128), dtype=f),
    }


def kernel(x, cos, sin, wq, bq, wk, bk, wv, bv, wo, cache_k, cache_v, start_pos,
           _trace=False):
    assert int(start_pos) == MAXLEN - 1
    if "nc" not in _CACHE:
        _CACHE["nc"] = _build()
    nc = _CACHE["nc"]

    args = [np.asarray(a) for a in
            (x, cos, sin, wq, bq, wk, bk, wv, bv, wo, cache_k, cache_v)]
    in_maps = []
    for core in range(NCORES):
        g, q = core % 2, core // 2
        in_maps.append(_prep_core(g, q, *args))

    res = bass_utils.run_bass_kernel_spmd(
        nc, in_maps, core_ids=list(range(NCORES)), trace=_trace)
    outs = [r["out"] for r in res.results]

    full = np.zeros((B, H), dtype=np.float32)
    for q in range(4):
        full[q * BL:(q + 1) * BL] = outs[2 * q] + outs[2 * q + 1]
    if _trace:
        kernel.last_results = res
    return full.reshape(B, 1, H)
